# revision 1
# baseline (speedup 1.0000x reference)
"""GCNConv-local Trainium2 kernel (8 NeuronCores, SPMD).

Math (reference):
    deg_i = 1 + #valid(edge_index[i]);  isd = deg^-0.5
    h = (x @ W.T) * isd
    out_i = (sum_d h[e_id] + h_i) * isd_i

Reformulated so the 256-wide matmul happens AFTER the neighbor sum, on only
the local shard (weight application commutes with the row-sum):
    xs_j  = isd_j * x_j                      (full table, built per core)
    y_i   = xs_i + sum_d xs[e_id]            (gather-accumulate, pad slots skipped)
    out_i = isd_i * (y_i @ W.T)

Sharding: nodes split contiguously across the 8 cores; x/edge-derived index
table replicated so no collectives are needed. The gather runs as
indirect-DMA accumulate chains ([128,1] row-gathers with CCE add + OOB skip),
which is bound by the Q7 SWDGE descriptor rate; everything else (build pass,
reduces, PE transposes + matmuls, stores) overlaps under it.
"""

import sys

if "/opt/trn_rl_repo" not in sys.path:
    sys.path.insert(0, "/opt/trn_rl_repo")

import numpy as np

import concourse.bass as bass
import concourse.mybir as mybir
from concourse.bass import IndirectOffsetOnAxis
from concourse.masks import make_identity
from concourse.tile import TileContext, add_dep_helper

P = 128
D = 256
MAXD = 16
MAXS = 17  # gather slots: col 0 = self, 1..16 = neighbors
NCORES = 8

F32 = mybir.dt.float32
BF16 = mybir.dt.bfloat16
I32 = mybir.dt.int32

XS_DT = BF16  # gathered-table dtype (bf16 halves build-write + gather traffic)


# ---------------------------------------------------------------------------
# walrus workaround: this image's walrus rejects >1-2 sync waits on one
# instruction. Split the Tile tail-drain across single-wait NOPs and hoist
# excess waits from every instruction onto preceding same-engine NOPs.
# ---------------------------------------------------------------------------

def _install_tile_fix():
    import bass_rust
    import concourse.tile as tile_mod
    from concourse.tile import TileContext as TC

    def _split_drain_and_barrier(self, tick_clock, wait_clock):
        gc = tick_clock.global_clock
        for i, t in enumerate(list(gc)):
            if t > 0:
                vc_l = [0] * len(list(gc))
                vc_l[i] = t
                nop = self.nc.sync.nop(nofuse=True, hint=f"drain_wait_{i}")
                wait_clock.add_sem_waits(
                    nop.ins,
                    tile_mod.ScopedClock({None: bass_rust.VectorClock(vc_l)}),
                )
        self.nc.sync.drain()
        self.nc.all_engine_barrier()
        assert self.sems is not None
        popped = self.nc._tile_sem_poison_stack.pop()
        assert popped is self._sem_poison
        self.nc.clear_and_free_semaphores(list(self.sems.allocated().values()))
        self.nc.all_engine_barrier()

    TC._drain_and_barrier = _split_drain_and_barrier


_install_tile_fix()

_MAX_WAITS = 1


def _fix_sync_waits(nc):
    n_fixed = 0
    for fn in nc.m.functions:
        for bb in fn.blocks:
            new_insts = []
            for inst in bb.instructions:
                si = inst.sync_info
                if si is not None and si.on_wait and len(si.on_wait) > _MAX_WAITS:
                    waits = list(si.on_wait)
                    keep = waits[-_MAX_WAITS:]
                    extra = waits[:-_MAX_WAITS]
                    for i in range(0, len(extra), _MAX_WAITS):
                        chunk = extra[i : i + _MAX_WAITS]
                        nop = mybir.InstNoOp(
                            name=nc.get_next_instruction_name(),
                            engine=inst.engine,
                            ins=[],
                            outs=[],
                            sync_info=mybir.SyncInfo(on_wait=chunk, on_update=[]),
                            bass_nofuse=True,
                            text_hint="split_wait",
                        )
                        nc.register_instruction(nop)
                        new_insts.append(nop)
                    si.on_wait = keep
                    n_fixed += 1
                new_insts.append(inst)
            bb.instructions[:] = new_insts
    return n_fixed


# ---------------------------------------------------------------------------
# kernel builder (one SPMD module; per-core data arrives via in_maps)
# ---------------------------------------------------------------------------

def build_nc(npad, acc_bufs=8):
    """npad: padded node count, multiple of 128*NCORES."""
    nl = npad // NCORES          # nodes per core
    t_shard = nl // P            # shard tiles per core
    t_full = npad // P           # build tiles (full table)
    sup = 8                      # slab rows per super-DMA
    assert (npad // P) % sup == 0
    ct = npad // P // 8          # isd chunk (slab rows) -> 8 chunks

    nc = bass.Bass("TRN2")
    x = nc.dram_tensor("x", [npad, D], F32, kind="ExternalInput")
    gidx = nc.dram_tensor("gidx", [npad, MAXS], I32, kind="ExternalInput")
    sgidx = nc.dram_tensor("sgidx", [nl, MAXS], I32, kind="ExternalInput")
    wt = nc.dram_tensor("wt", [D, D], F32, kind="ExternalInput")
    out = nc.dram_tensor("out", [nl, D], F32, kind="ExternalOutput")
    xs = nc.dram_tensor("xs", [npad + P, D], XS_DT)

    with TileContext(nc) as tc:
        with (
            tc.tile_pool(name="const", bufs=1) as cpool,
            tc.tile_pool(name="deg", bufs=2) as dpool,
            tc.tile_pool(name="build", bufs=2) as bpool,
            tc.tile_pool(name="accp", bufs=1) as apool,
            tc.tile_pool(name="gat", bufs=4) as gpool,
            tc.tile_pool(name="psum", bufs=4, space="PSUM") as ppool,
        ):
            # --- constants -------------------------------------------------
            ident = cpool.tile([P, P], BF16, name="ident")
            make_identity(nc, ident[:])

            wtf = cpool.tile([P, 2, D], F32, name="wtf")
            nc.sync.dma_start(wtf[:], wt[:].rearrange("(c p) d -> p c d", p=P))
            wtb = cpool.tile([P, 2, D], BF16, name="wtb")
            nc.vector.tensor_copy(wtb[:], wtf[:])

            # --- full-table isd [P, rows_pp] (partition-major slabs) -------
            rows_pp = npad // P          # rows per partition
            isd = cpool.tile([P, rows_pp], F32, name="isd")
            gv = gidx[:].rearrange("(p r) s -> p r s", p=P)
            for c0 in range(0, rows_pp, ct):
                gt = dpool.tile([P, ct, MAXS], I32, name="gt")
                nc.sync.dma_start(gt[:], gv[:, c0 : c0 + ct, :])
                m = dpool.tile([P, ct, MAXS], F32, name="m")
                nc.vector.tensor_scalar(
                    m[:], gt[:], npad - 1, None, op0=mybir.AluOpType.is_le
                )
                dg = dpool.tile([P, ct], F32, name="dg")
                nc.vector.reduce_sum(dg[:], m[:], axis=mybir.AxisListType.X)
                nc.scalar.activation(
                    dg[:], dg[:], mybir.ActivationFunctionType.Sqrt
                )
                nc.vector.reciprocal(isd[:, c0 : c0 + ct], dg[:])

            # --- shard isd [P, t_shard] + resident shard indices -----------
            sg = cpool.tile([P, t_shard, MAXS], I32, name="sg")
            sgc = 7 if t_shard % 7 == 0 else (2 if t_shard % 2 == 0 else 1)
            for c0 in range(0, t_shard, sgc):
                nc.sync.dma_start(
                    sg[:, c0 : c0 + sgc, :],
                    sgidx[c0 * P : (c0 + sgc) * P, :].rearrange(
                        "(t p) s -> p t s", p=P
                    ),
                )
            isd_sh = cpool.tile([P, t_shard], F32, name="isd_sh")
            msh = dpool.tile([P, t_shard, MAXS], F32, name="msh")
            nc.vector.tensor_scalar(
                msh[:],
                sg[:],
                npad - 1,
                None,
                op0=mybir.AluOpType.is_le,
            )
            dgs = dpool.tile([P, t_shard], F32, name="dgs")
            nc.vector.reduce_sum(dgs[:], msh[:], axis=mybir.AxisListType.X)
            nc.scalar.activation(
                dgs[:], dgs[:], mybir.ActivationFunctionType.Sqrt
            )
            nc.vector.reciprocal(isd_sh[:], dgs[:])

            # --- phase 1: xs = x * isd (full table, p-major slabs) ---------
            xv = x[:].rearrange("(p r) d -> p r d", p=P)
            xsv = xs[0:npad, :].rearrange("(p r) d -> p r d", p=P)
            for g in range(rows_pp // sup):
                xt = bpool.tile([P, sup, D], F32, name="xt")
                nc.sync.dma_start(xt[:], xv[:, g * sup : (g + 1) * sup, :])
                xst = bpool.tile([P, sup, D], XS_DT, name="xst")
                for s in range(sup):
                    nc.vector.tensor_scalar_mul(
                        xst[:, s, :], xt[:, s, :], isd[:, g * sup + s : g * sup + s + 1]
                    )
                nc.sync.dma_start(xsv[:, g * sup : (g + 1) * sup, :], xst[:])
            zt = bpool.tile([P, D], XS_DT, name="zt")
            nc.vector.memset(zt[:], 0.0)
            nc.sync.dma_start(xs[npad : npad + P, :], zt[:])

            # --- phase 2: pure-SWDGE gather chains (slot 0 = self, bypass) --
            accs = []
            last_g = []
            for t in range(t_shard):
                acc = apool.tile([P, D], F32, name=f"acc{t}", tag=f"acc{t}")
                accs.append(acc)
                inst = None
                for s in range(MAXS):
                    inst = nc.gpsimd.indirect_dma_start(
                        out=acc[:],
                        out_offset=None,
                        in_=xs[:],
                        in_offset=IndirectOffsetOnAxis(
                            ap=sg[:, t, s : s + 1], axis=0
                        ),
                        compute_op=(
                            mybir.AluOpType.bypass
                            if s == 0
                            else mybir.AluOpType.add
                        ),
                    )
                last_g.append(inst.ins)

            # barrier: no DVE work may overlap the SWDGE gather phase
            # (SWDGE descriptor rings share SBUF ports with DVE)
            joint = nc.sync.nop(nofuse=True, hint="gather_join")
            for gi_inst in last_g:
                add_dep_helper(joint.ins, gi_inst, reason="join gather chains")

            # --- phase 3: scale + transpose + matmul + store per tile ------
            for t in range(t_shard):
                yb = gpool.tile([P, D], BF16, name="yb")
                i0 = nc.vector.tensor_scalar_mul(
                    yb[:], accs[t][:], isd_sh[:, t : t + 1]
                )
                add_dep_helper(i0.ins, joint.ins, reason="tail after gathers")
                ytt = gpool.tile([P, 2, P], BF16, name="ytt")
                for ci in range(2):
                    pt = ppool.tile([P, P], BF16, name="pt")
                    nc.tensor.transpose(pt[:], yb[:, ci * P : (ci + 1) * P], ident[:])
                    nc.vector.tensor_copy(ytt[:, ci, :], pt[:])
                po = ppool.tile([P, D], F32, name="po")
                for ci in range(2):
                    nc.tensor.matmul(
                        po[:],
                        ytt[:, ci, :],
                        wtb[:, ci, :],
                        start=(ci == 0),
                        stop=(ci == 1),
                    )
                ot = gpool.tile([P, D], F32, name="ot")
                nc.vector.tensor_copy(ot[:], po[:])
                nc.sync.dma_start(out[t * P : (t + 1) * P, :], ot[:])

    _fix_sync_waits(nc)
    return nc


# ---------------------------------------------------------------------------
# host entry point
# ---------------------------------------------------------------------------

def _prep(x, edge_index, W):
    x = np.ascontiguousarray(np.asarray(x, dtype=np.float32))
    ei = np.asarray(edge_index)
    W = np.ascontiguousarray(np.asarray(W, dtype=np.float32))
    n = x.shape[0]
    npad = -(-n // (P * NCORES)) * (P * NCORES)
    nl = npad // NCORES

    xp = np.zeros((npad, D), np.float32)
    xp[:n] = x
    gi = np.full((npad, MAXS), npad, np.int32)  # sentinel = npad (skipped)
    gi[:, 0] = np.arange(npad, dtype=np.int32)  # slot 0 = self (bypass init)
    e = ei.astype(np.int64)
    gi[:n, 1:] = np.where(e < 0, npad, e).astype(np.int32)
    wt = np.ascontiguousarray(W.T)

    in_maps = []
    for c in range(NCORES):
        in_maps.append(
            {
                "x": xp,
                "gidx": gi,
                "sgidx": np.ascontiguousarray(gi[c * nl : (c + 1) * nl]),
                "wt": wt,
            }
        )
    return npad, n, in_maps


def kernel(x, edge_index, W, trace=False):
    from concourse.bass_utils import run_bass_kernel_spmd

    npad, n, in_maps = _prep(x, edge_index, W)
    nc = build_nc(npad)
    res = run_bass_kernel_spmd(
        nc, in_maps, core_ids=list(range(NCORES)), trace=trace
    )
    out = np.concatenate([res.results[c]["out"] for c in range(NCORES)], axis=0)
    kernel.last_exec_time_ns = res.exec_time_ns
    kernel.last_results = res
    return out[:n].astype(np.float32)


kernel.last_exec_time_ns = None



# revision 6
# speedup vs baseline: 1.3770x; 1.3770x over previous
"""GCNConv-local Trainium2 kernel (8 NeuronCores, SPMD).

Math (reference):
    deg_i = 1 + #valid(edge_index[i]);  isd = deg^-0.5
    h = (x @ W.T) * isd
    out_i = (sum_d h[e_id] + h_i) * isd_i

Reformulated so the 256-wide matmul happens AFTER the neighbor sum, on only
the local shard (weight application commutes with the row-sum):
    xs_j  = isd_j * x_j                      (full table, built per core)
    y_i   = xs_i + sum_d xs[e_id]            (gather + slot-reduce)
    out_i = isd_i * (y_i @ W.T)

Sharding: nodes split contiguously across the 8 cores; the scaled table is
replicated so no collectives are needed. The neighbor gather runs as
UNCHAINED per-slot indirect DMAs (bypass into disjoint 512B slot ranges of
a [128, 17*256] bf16 tile; the indirect1d ucode only supports one offset
per partition per instruction), round-robined across 4 SWDGE queues so
descriptor generation pipelines. The self slot is a direct strided DMA.
Slot reduction is a log-tree of wide contiguous DVE adds; the per-tile
transpose+matmul tail and the build pass overlap under the gather DMA.
"""

import sys

if "/opt/trn_rl_repo" not in sys.path:
    sys.path.insert(0, "/opt/trn_rl_repo")

import numpy as np

import concourse.bass as bass
import concourse.mybir as mybir
from concourse.bass import IndirectOffsetOnAxis
from concourse.masks import make_identity
from concourse.tile import TileContext

P = 128
D = 256
MAXD = 16
MAXS = 17  # gather slots: cols 0..15 = neighbors, col 16 = self
NCORES = 8

F32 = mybir.dt.float32
BF16 = mybir.dt.bfloat16
I32 = mybir.dt.int32

XS_DT = BF16  # gathered-table dtype (bf16 halves build-write + gather traffic)


# ---------------------------------------------------------------------------
# walrus workaround: this image's walrus rejects >1-2 sync waits on one
# instruction. Split the Tile tail-drain across single-wait NOPs and hoist
# excess waits from every instruction onto preceding same-engine NOPs.
# ---------------------------------------------------------------------------

def _install_tile_fix():
    import bass_rust
    import concourse.tile as tile_mod
    from concourse.tile import TileContext as TC

    def _split_drain_and_barrier(self, tick_clock, wait_clock):
        gc = tick_clock.global_clock
        for i, t in enumerate(list(gc)):
            if t > 0:
                vc_l = [0] * len(list(gc))
                vc_l[i] = t
                nop = self.nc.sync.nop(nofuse=True, hint=f"drain_wait_{i}")
                wait_clock.add_sem_waits(
                    nop.ins,
                    tile_mod.ScopedClock({None: bass_rust.VectorClock(vc_l)}),
                )
        self.nc.sync.drain()
        self.nc.all_engine_barrier()
        assert self.sems is not None
        popped = self.nc._tile_sem_poison_stack.pop()
        assert popped is self._sem_poison
        self.nc.clear_and_free_semaphores(list(self.sems.allocated().values()))
        self.nc.all_engine_barrier()

    TC._drain_and_barrier = _split_drain_and_barrier


_install_tile_fix()

_MAX_WAITS = 1


def _fix_sync_waits(nc):
    n_fixed = 0
    for fn in nc.m.functions:
        for bb in fn.blocks:
            new_insts = []
            for inst in bb.instructions:
                si = inst.sync_info
                if si is not None and si.on_wait and len(si.on_wait) > _MAX_WAITS:
                    waits = list(si.on_wait)
                    keep = waits[-_MAX_WAITS:]
                    extra = waits[:-_MAX_WAITS]
                    for i in range(0, len(extra), _MAX_WAITS):
                        chunk = extra[i : i + _MAX_WAITS]
                        nop = mybir.InstNoOp(
                            name=nc.get_next_instruction_name(),
                            engine=inst.engine,
                            ins=[],
                            outs=[],
                            sync_info=mybir.SyncInfo(on_wait=chunk, on_update=[]),
                            bass_nofuse=True,
                            text_hint="split_wait",
                        )
                        nc.register_instruction(nop)
                        new_insts.append(nop)
                    si.on_wait = keep
                    n_fixed += 1
                new_insts.append(inst)
            bb.instructions[:] = new_insts
    return n_fixed


# ---------------------------------------------------------------------------
# kernel builder (one SPMD module; per-core data arrives via in_maps)
# ---------------------------------------------------------------------------

NQ = 4  # SWDGE queues for the gather phase


def build_nc(npad):
    """npad: padded node count, multiple of 128*NCORES."""
    nl = npad // NCORES          # nodes per core
    t_shard = nl // P            # shard tiles per core
    sup = 8                      # slab rows per super-DMA
    assert (npad // P) % sup == 0
    ct = npad // P // 8          # isd chunk (slab rows) -> 8 chunks

    nc = bass.Bass("TRN2", num_swdge_queues=NQ)
    x = nc.dram_tensor("x", [npad, D], F32, kind="ExternalInput")
    gidx = nc.dram_tensor("gidx", [npad, MAXS], I32, kind="ExternalInput")
    sgidx = nc.dram_tensor("sgidx", [nl, MAXS], I32, kind="ExternalInput")
    wt = nc.dram_tensor("wt", [D, D], F32, kind="ExternalInput")
    out = nc.dram_tensor("out", [nl, D], F32, kind="ExternalOutput")
    xs = nc.dram_tensor("xs", [npad + P, D], XS_DT)

    with TileContext(nc) as tc:
        with (
            tc.tile_pool(name="const", bufs=1) as cpool,
            tc.tile_pool(name="deg", bufs=2) as dpool,
            tc.tile_pool(name="build", bufs=2) as bpool,
            tc.tile_pool(name="gat", bufs=4) as gpool,
            tc.tile_pool(name="psum", bufs=4, space="PSUM") as ppool,
        ):
            # --- constants -------------------------------------------------
            ident = cpool.tile([P, P], BF16, name="ident")
            make_identity(nc, ident[:])

            wtf = cpool.tile([P, 2, D], F32, name="wtf")
            nc.sync.dma_start(wtf[:], wt[:].rearrange("(c p) d -> p c d", p=P))
            wtb = cpool.tile([P, 2, D], BF16, name="wtb")
            nc.vector.tensor_copy(wtb[:], wtf[:])

            # --- full-table isd [P, rows_pp] (partition-major slabs) -------
            rows_pp = npad // P          # rows per partition
            isd = cpool.tile([P, rows_pp], F32, name="isd")
            gv = gidx[:].rearrange("(p r) s -> p r s", p=P)
            for c0 in range(0, rows_pp, ct):
                gt = dpool.tile([P, ct, MAXS], I32, name="gt")
                nc.sync.dma_start(gt[:], gv[:, c0 : c0 + ct, :])
                m = dpool.tile([P, ct, MAXS], F32, name="m")
                nc.vector.tensor_scalar(
                    m[:], gt[:], npad - 1, None, op0=mybir.AluOpType.is_le
                )
                dg = dpool.tile([P, ct], F32, name="dg")
                nc.vector.reduce_sum(dg[:], m[:], axis=mybir.AxisListType.X)
                nc.scalar.activation(
                    dg[:], dg[:], mybir.ActivationFunctionType.Sqrt
                )
                nc.vector.reciprocal(isd[:, c0 : c0 + ct], dg[:])

            # --- shard isd [P, t_shard] + resident shard indices -----------
            sg = cpool.tile([P, t_shard, MAXS], I32, name="sg")
            sgc = 7 if t_shard % 7 == 0 else (2 if t_shard % 2 == 0 else 1)
            for c0 in range(0, t_shard, sgc):
                nc.sync.dma_start(
                    sg[:, c0 : c0 + sgc, :],
                    sgidx[c0 * P : (c0 + sgc) * P, :].rearrange(
                        "(t p) s -> p t s", p=P
                    ),
                )
            isd_sh = cpool.tile([P, t_shard], F32, name="isd_sh")
            msh = dpool.tile([P, t_shard, MAXS], F32, name="msh")
            nc.vector.tensor_scalar(
                msh[:],
                sg[:],
                npad - 1,
                None,
                op0=mybir.AluOpType.is_le,
            )
            dgs = dpool.tile([P, t_shard], F32, name="dgs")
            nc.vector.reduce_sum(dgs[:], msh[:], axis=mybir.AxisListType.X)
            nc.scalar.activation(
                dgs[:], dgs[:], mybir.ActivationFunctionType.Sqrt
            )
            nc.vector.reciprocal(isd_sh[:], dgs[:])

            # --- phase 1: xs = x * isd (full table, p-major slabs) ---------
            xv = x[:].rearrange("(p r) d -> p r d", p=P)
            xsv = xs[0:npad, :].rearrange("(p r) d -> p r d", p=P)
            for g in range(rows_pp // sup):
                xt = bpool.tile([P, sup, D], F32, name="xt")
                nc.sync.dma_start(xt[:], xv[:, g * sup : (g + 1) * sup, :])
                xst = bpool.tile([P, sup, D], XS_DT, name="xst")
                for s in range(sup):
                    nc.vector.tensor_scalar_mul(
                        xst[:, s, :], xt[:, s, :], isd[:, g * sup + s : g * sup + s + 1]
                    )
                nc.sync.dma_start(xsv[:, g * sup : (g + 1) * sup, :], xst[:])
            zt = bpool.tile([P, D], XS_DT, name="zt")
            nc.vector.memset(zt[:], 0.0)
            nc.sync.dma_start(xs[npad : npad + P, :], zt[:])

            # --- phase 2: per-slot unchained gathers + tree reduce ---------
            # gf[p, s*256:(s+1)*256] = xs[sg[p, t, s]]  (sentinel -> zero row)
            gq = 0
            for t in range(t_shard):
                gf = gpool.tile([P, MAXS * D], XS_DT, name="gf")
                for s in range(MAXS):
                    bi = nc.gpsimd.indirect_dma_start(
                        out=gf[:, s * D : (s + 1) * D],
                        out_offset=None,
                        in_=xs[:],
                        in_offset=IndirectOffsetOnAxis(
                            ap=sg[:, t, s : s + 1], axis=0
                        ),
                        compute_op=mybir.AluOpType.bypass,
                    )
                    q = gq % NQ
                    if q:
                        bi.ins.queue = f"qPoolDynamic{q}"
                    gq += 1
                # tree-reduce 16 neighbor slots (wide contiguous bf16 adds),
                # then add the self slot (col 16)
                u1 = gpool.tile([P, 8 * D], XS_DT, name="u1")
                nc.vector.tensor_add(u1[:], gf[:, 0 : 8 * D], gf[:, 8 * D : 16 * D])
                u2 = gpool.tile([P, 4 * D], XS_DT, name="u2")
                nc.vector.tensor_add(u2[:], u1[:, 0 : 4 * D], u1[:, 4 * D : 8 * D])
                u3 = gpool.tile([P, 2 * D], XS_DT, name="u3")
                nc.vector.tensor_add(u3[:], u2[:, 0 : 2 * D], u2[:, 2 * D : 4 * D])
                u4 = gpool.tile([P, D], XS_DT, name="u4")
                nc.vector.tensor_add(u4[:], u3[:, 0:D], u3[:, D : 2 * D])
                yb = gpool.tile([P, D], BF16, name="yb")
                nc.vector.tensor_add(yb[:], u4[:], gf[:, 16 * D : 17 * D])

                # transpose + matmul + scale + store
                ytt = gpool.tile([P, 2, P], BF16, name="ytt")
                for ci in range(2):
                    pt = ppool.tile([P, P], BF16, name="pt")
                    nc.tensor.transpose(pt[:], yb[:, ci * P : (ci + 1) * P], ident[:])
                    nc.vector.tensor_copy(ytt[:, ci, :], pt[:])
                po = ppool.tile([P, D], F32, name="po")
                for ci in range(2):
                    nc.tensor.matmul(
                        po[:],
                        ytt[:, ci, :],
                        wtb[:, ci, :],
                        start=(ci == 0),
                        stop=(ci == 1),
                    )
                ot = gpool.tile([P, D], F32, name="ot")
                nc.vector.tensor_scalar_mul(ot[:], po[:], isd_sh[:, t : t + 1])
                nc.sync.dma_start(out[t * P : (t + 1) * P, :], ot[:])

    _fix_sync_waits(nc)
    return nc


# ---------------------------------------------------------------------------
# host entry point
# ---------------------------------------------------------------------------

def _prep(x, edge_index, W):
    x = np.ascontiguousarray(np.asarray(x, dtype=np.float32))
    ei = np.asarray(edge_index)
    W = np.ascontiguousarray(np.asarray(W, dtype=np.float32))
    n = x.shape[0]
    npad = -(-n // (P * NCORES)) * (P * NCORES)
    nl = npad // NCORES

    xp = np.zeros((npad, D), np.float32)
    xp[:n] = x
    gi = np.full((npad, MAXS), npad, np.int32)  # sentinel = npad (zero row)
    gi[:, MAXS - 1] = np.arange(npad, dtype=np.int32)  # last slot = self
    e = ei.astype(np.int64)
    gi[:n, : MAXS - 1] = np.where(e < 0, npad, e).astype(np.int32)
    wt = np.ascontiguousarray(W.T)

    in_maps = []
    for c in range(NCORES):
        in_maps.append(
            {
                "x": xp,
                "gidx": gi,
                "sgidx": np.ascontiguousarray(gi[c * nl : (c + 1) * nl]),
                "wt": wt,
            }
        )
    return npad, n, in_maps


def kernel(x, edge_index, W, trace=False):
    from concourse.bass_utils import run_bass_kernel_spmd

    npad, n, in_maps = _prep(x, edge_index, W)
    nc = build_nc(npad)
    res = run_bass_kernel_spmd(
        nc, in_maps, core_ids=list(range(NCORES)), trace=trace
    )
    out = np.concatenate([res.results[c]["out"] for c in range(NCORES)], axis=0)
    kernel.last_exec_time_ns = res.exec_time_ns
    kernel.last_results = res
    return out[:n].astype(np.float32)


kernel.last_exec_time_ns = None


# revision 7
# speedup vs baseline: 1.8112x; 1.3154x over previous
"""GCNConv-local Trainium2 kernel (8 NeuronCores, SPMD).

Math (reference):
    deg_i = 1 + #valid(edge_index[i]);  isd = deg^-0.5
    h = (x @ W.T) * isd
    out_i = (sum_d h[e_id] + h_i) * isd_i

Reformulated so the 256-wide matmul happens AFTER the neighbor sum, on only
the local shard (weight application commutes with the row-sum):
    xs_j  = isd_j * x_j                      (full table, built per core)
    y_i   = xs_i + sum_d xs[e_id]            (gather + slot accumulation)
    out_i = isd_i * (y_i @ W.T)

Sharding: nodes split contiguously across the 8 cores; the scaled table is
replicated so no collectives are needed.

The gather is bound by the Pool engine's SWDGE descriptor-generation ucode
(~1.1us per indirect DMA, 128 rows each; the indirect1d ucode supports only
one offset per partition per instruction). To minimize and hide that cost:
  - host pre-pass sorts each node's 17 slots ascending (sentinels compact to
    the back and truncate), sorts nodes by degree so each 128-row tile needs
    only max-degree-in-tile gather slots (~26% fewer instructions), and
    un-permutes the rows on the way out;
  - the table is built in global-row-prefix order and each gather declares a
    shrunken source extent + explicit dep on the covering build slab, so
    early slot rounds (small sorted indices) start while the build is still
    streaming;
  - gathers land in a small ring and are folded into per-tile bf16
    accumulators; the transpose+matmul tail runs as soon as a tile's last
    slot round completes.
"""

import sys

if "/opt/trn_rl_repo" not in sys.path:
    sys.path.insert(0, "/opt/trn_rl_repo")

import numpy as np

import concourse.bass as bass
import concourse.mybir as mybir
from concourse.bass import IndirectOffsetOnAxis
from concourse.masks import make_identity
from concourse.tile import TileContext, add_dep_helper

P = 128
D = 256
MAXD = 16
MAXS = 17  # slots per node: 16 neighbors + self (sorted, sentinel-compacted)
NCORES = 8
SLAB = 1024  # rows per build slab (prefix order)

F32 = mybir.dt.float32
BF16 = mybir.dt.bfloat16
I32 = mybir.dt.int32

XS_DT = BF16  # gathered-table dtype (bf16 halves build-write + gather traffic)


# ---------------------------------------------------------------------------
# walrus workaround: this image's walrus rejects >1-2 sync waits on one
# instruction. Split the Tile tail-drain across single-wait NOPs and hoist
# excess waits from every instruction onto preceding same-engine NOPs.
# ---------------------------------------------------------------------------

def _install_tile_fix():
    import bass_rust
    import concourse.tile as tile_mod
    from concourse.tile import TileContext as TC

    def _split_drain_and_barrier(self, tick_clock, wait_clock):
        gc = tick_clock.global_clock
        for i, t in enumerate(list(gc)):
            if t > 0:
                vc_l = [0] * len(list(gc))
                vc_l[i] = t
                nop = self.nc.sync.nop(nofuse=True, hint=f"drain_wait_{i}")
                wait_clock.add_sem_waits(
                    nop.ins,
                    tile_mod.ScopedClock({None: bass_rust.VectorClock(vc_l)}),
                )
        self.nc.sync.drain()
        self.nc.all_engine_barrier()
        assert self.sems is not None
        popped = self.nc._tile_sem_poison_stack.pop()
        assert popped is self._sem_poison
        self.nc.clear_and_free_semaphores(list(self.sems.allocated().values()))
        self.nc.all_engine_barrier()

    TC._drain_and_barrier = _split_drain_and_barrier


_install_tile_fix()

_MAX_WAITS = 1


def _fix_sync_waits(nc):
    n_fixed = 0
    for fn in nc.m.functions:
        for bb in fn.blocks:
            new_insts = []
            for inst in bb.instructions:
                si = inst.sync_info
                if si is not None and si.on_wait and len(si.on_wait) > _MAX_WAITS:
                    waits = list(si.on_wait)
                    keep = waits[-_MAX_WAITS:]
                    extra = waits[:-_MAX_WAITS]
                    for i in range(0, len(extra), _MAX_WAITS):
                        chunk = extra[i : i + _MAX_WAITS]
                        nop = mybir.InstNoOp(
                            name=nc.get_next_instruction_name(),
                            engine=inst.engine,
                            ins=[],
                            outs=[],
                            sync_info=mybir.SyncInfo(on_wait=chunk, on_update=[]),
                            bass_nofuse=True,
                            text_hint="split_wait",
                        )
                        nc.register_instruction(nop)
                        new_insts.append(nop)
                    si.on_wait = keep
                    n_fixed += 1
                new_insts.append(inst)
            bb.instructions[:] = new_insts
    return n_fixed


# ---------------------------------------------------------------------------
# kernel builder (one SPMD module; per-core data arrives via in_maps)
# ---------------------------------------------------------------------------

def build_nc(npad, k_tiles, col_max):
    """npad: padded node count (multiple of 128*NCORES).
    k_tiles[t]: gather slots needed for tile t (max across cores).
    col_max[t][s]: max table row touched by column (t, s) across cores
                   (0 = sentinel/zero row)."""
    nl = npad // NCORES          # nodes per core
    t_shard = nl // P            # shard tiles per core
    sup = SLAB // P              # rows-of-128 per build slab
    n_slab = npad // SLAB
    ct = npad // P // 8          # isd chunk -> 8 chunks

    nc = bass.Bass("TRN2")
    x = nc.dram_tensor("x", [npad, D], F32, kind="ExternalInput")
    gidx = nc.dram_tensor("gidx", [npad, MAXS], I32, kind="ExternalInput")
    sgidx = nc.dram_tensor("sgidx", [nl, MAXS], I32, kind="ExternalInput")
    wt = nc.dram_tensor("wt", [D, D], F32, kind="ExternalInput")
    out = nc.dram_tensor("out", [nl, D], F32, kind="ExternalOutput")
    # row 0 = zero row (sentinel target); node n lives at row n+1
    xs = nc.dram_tensor("xs", [npad + P, D], XS_DT)

    with TileContext(nc) as tc:
        with (
            tc.tile_pool(name="const", bufs=1) as cpool,
            tc.tile_pool(name="deg", bufs=2) as dpool,
            tc.tile_pool(name="build", bufs=3) as bpool,
            tc.tile_pool(name="accp", bufs=1) as apool,
            tc.tile_pool(name="ring", bufs=24) as rpool,
            tc.tile_pool(name="tail", bufs=4) as tpool,
            tc.tile_pool(name="psum", bufs=4, space="PSUM") as ppool,
        ):
            # --- constants -------------------------------------------------
            ident = cpool.tile([P, P], BF16, name="ident")
            make_identity(nc, ident[:])

            wtf = cpool.tile([P, 2, D], F32, name="wtf")
            nc.sync.dma_start(wtf[:], wt[:].rearrange("(c p) d -> p c d", p=P))
            wtb = cpool.tile([P, 2, D], BF16, name="wtb")
            nc.vector.tensor_copy(wtb[:], wtf[:])

            # zero row (sentinel target) -- written before any build slab
            zt = bpool.tile([P, D], XS_DT, name="zt")
            nc.vector.memset(zt[:], 0.0)
            zrow = nc.sync.dma_start(xs[0:P, :], zt[:])

            # --- full-table isd [P, rows_pp], node n = r*128 + p -----------
            rows_pp = npad // P
            isd = cpool.tile([P, rows_pp], F32, name="isd")
            gv = gidx[:].rearrange("(r p) s -> p r s", p=P)
            for c0 in range(0, rows_pp, ct):
                gt = dpool.tile([P, ct, MAXS], I32, name="gt")
                nc.sync.dma_start(gt[:], gv[:, c0 : c0 + ct, :])
                m = dpool.tile([P, ct, MAXS], F32, name="m")
                nc.vector.tensor_scalar(
                    m[:], gt[:], npad - 1, None, op0=mybir.AluOpType.is_le
                )
                dg = dpool.tile([P, ct], F32, name="dg")
                nc.vector.reduce_sum(dg[:], m[:], axis=mybir.AxisListType.X)
                nc.scalar.activation(
                    dg[:], dg[:], mybir.ActivationFunctionType.Sqrt
                )
                nc.vector.reciprocal(isd[:, c0 : c0 + ct], dg[:])

            # --- shard isd [P, t_shard] + sorted shard indices -------------
            # sgidx values: 0 = sentinel (zero row), else table row (node+1)
            sg = cpool.tile([P, t_shard, MAXS], I32, name="sg")
            sgc = 7 if t_shard % 7 == 0 else (2 if t_shard % 2 == 0 else 1)
            for c0 in range(0, t_shard, sgc):
                nc.sync.dma_start(
                    sg[:, c0 : c0 + sgc, :],
                    sgidx[c0 * P : (c0 + sgc) * P, :].rearrange(
                        "(t p) s -> p t s", p=P
                    ),
                )
            isd_sh = cpool.tile([P, t_shard], F32, name="isd_sh")
            msh = dpool.tile([P, t_shard, MAXS], F32, name="msh")
            nc.vector.tensor_scalar(
                msh[:], sg[:], 1, None, op0=mybir.AluOpType.is_ge
            )
            dgs = dpool.tile([P, t_shard], F32, name="dgs")
            nc.vector.reduce_sum(dgs[:], msh[:], axis=mybir.AxisListType.X)
            nc.scalar.activation(
                dgs[:], dgs[:], mybir.ActivationFunctionType.Sqrt
            )
            nc.vector.reciprocal(isd_sh[:], dgs[:])

            # --- phase 1: xs[n+1] = x[n] * isd[n], global prefix order -----
            # slab g covers nodes [g*SLAB, (g+1)*SLAB) -> rows +1
            slab_store = []
            for g in range(n_slab):
                xt = bpool.tile([P, sup, D], F32, name="xt")
                nc.sync.dma_start(
                    xt[:],
                    x[g * SLAB : (g + 1) * SLAB, :].rearrange(
                        "(r p) d -> p r d", p=P
                    ),
                )
                xst = bpool.tile([P, sup, D], XS_DT, name="xst")
                for r in range(sup):
                    nc.vector.tensor_scalar_mul(
                        xst[:, r, :], xt[:, r, :],
                        isd[:, g * sup + r : g * sup + r + 1],
                    )
                st = nc.scalar.dma_start(
                    xs[1 + g * SLAB : 1 + (g + 1) * SLAB, :].rearrange(
                        "(r p) d -> p r d", p=P
                    ),
                    xst[:],
                )
                slab_store.append(st)

            # --- phase 2: gather rounds + per-tile accumulate --------------
            max_k = max(k_tiles)
            accs = [None] * t_shard
            for s in range(max_k):
                for t in range(t_shard):
                    if k_tiles[t] <= s:
                        continue
                    mv = col_max[t][s]
                    if mv <= 0:
                        lim, dep = P, zrow
                    else:
                        gsl = (mv - 1) // SLAB
                        lim, dep = 1 + (gsl + 1) * SLAB, slab_store[gsl]
                    gb = rpool.tile([P, D], XS_DT, name="gb")
                    bi = nc.gpsimd.indirect_dma_start(
                        out=gb[:],
                        out_offset=None,
                        in_=xs[0:lim, :],
                        in_offset=IndirectOffsetOnAxis(
                            ap=sg[:, t, s : s + 1], axis=0
                        ),
                        compute_op=mybir.AluOpType.bypass,
                    )
                    add_dep_helper(bi.ins, dep.ins, reason="xs prefix built")
                    if s == 0:
                        acc = apool.tile([P, D], XS_DT, name=f"acc{t}",
                                         tag=f"acc{t}")
                        accs[t] = acc
                        nc.vector.tensor_copy(acc[:], gb[:])
                    else:
                        nc.vector.tensor_add(accs[t][:], accs[t][:], gb[:])
                # tails for tiles whose last round just completed
                for t in range(t_shard):
                    if k_tiles[t] != s + 1:
                        continue
                    acc = accs[t]
                    ytt = tpool.tile([P, 2, P], BF16, name="ytt")
                    for ci in range(2):
                        pt = ppool.tile([P, P], BF16, name="pt")
                        nc.tensor.transpose(
                            pt[:], acc[:, ci * P : (ci + 1) * P], ident[:]
                        )
                        nc.vector.tensor_copy(ytt[:, ci, :], pt[:])
                    po = ppool.tile([P, D], F32, name="po")
                    for ci in range(2):
                        nc.tensor.matmul(
                            po[:],
                            ytt[:, ci, :],
                            wtb[:, ci, :],
                            start=(ci == 0),
                            stop=(ci == 1),
                        )
                    ot = tpool.tile([P, D], F32, name="ot")
                    nc.vector.tensor_scalar_mul(
                        ot[:], po[:], isd_sh[:, t : t + 1]
                    )
                    nc.sync.dma_start(out[t * P : (t + 1) * P, :], ot[:])

    _fix_sync_waits(nc)
    return nc


# ---------------------------------------------------------------------------
# host entry point
# ---------------------------------------------------------------------------

def _prep(x, edge_index, W):
    x = np.ascontiguousarray(np.asarray(x, dtype=np.float32))
    ei = np.asarray(edge_index)
    W = np.ascontiguousarray(np.asarray(W, dtype=np.float32))
    n = x.shape[0]
    npad = -(-n // (P * NCORES)) * (P * NCORES)
    nl = npad // NCORES
    t_shard = nl // P

    xp = np.zeros((npad, D), np.float32)
    xp[:n] = x

    # full-table slot lists (UNSORTED; sentinel npad; self in last slot) --
    # only used on-device to compute isd for the build scaling
    gi = np.full((npad, MAXS), npad, np.int32)
    gi[:, MAXS - 1] = np.arange(npad, dtype=np.int32)
    e = ei.astype(np.int64)
    gi[:n, : MAXS - 1] = np.where(e < 0, npad, e).astype(np.int32)
    wt = np.ascontiguousarray(W.T)

    # per-core shard prep: degree sort, ascending slot sort, +1/0 remap ----
    deg_all = (gi < npad).sum(axis=1)  # includes self
    orders, sgidxs = [], []
    kc = np.zeros((NCORES, t_shard), np.int32)
    colmax = np.zeros((NCORES, t_shard, MAXS), np.int64)
    for c in range(NCORES):
        sh = gi[c * nl : (c + 1) * nl]
        dg = deg_all[c * nl : (c + 1) * nl]
        order = np.argsort(-dg, kind="stable")
        shs = np.sort(sh[order], axis=1)          # ascending; sentinels last
        vals = np.where(shs >= npad, 0, shs + 1).astype(np.int32)
        orders.append(order)
        sgidxs.append(np.ascontiguousarray(vals))
        dgo = dg[order].reshape(t_shard, P)
        kc[c] = dgo.max(axis=1)
        colmax[c] = vals.reshape(t_shard, P, MAXS).max(axis=1)

    k_tiles = kc.max(axis=0).tolist()
    col_max = colmax.max(axis=0).tolist()

    in_maps = []
    for c in range(NCORES):
        in_maps.append(
            {"x": xp, "gidx": gi, "sgidx": sgidxs[c], "wt": wt}
        )
    return npad, n, in_maps, orders, k_tiles, col_max


def kernel(x, edge_index, W, trace=False):
    from concourse.bass_utils import run_bass_kernel_spmd

    npad, n, in_maps, orders, k_tiles, col_max = _prep(x, edge_index, W)
    nl = npad // NCORES
    nc = build_nc(npad, k_tiles, col_max)
    res = run_bass_kernel_spmd(
        nc, in_maps, core_ids=list(range(NCORES)), trace=trace
    )
    out = np.empty((npad, D), np.float32)
    for c in range(NCORES):
        blk = out[c * nl : (c + 1) * nl]
        blk[orders[c]] = res.results[c]["out"]
    kernel.last_exec_time_ns = res.exec_time_ns
    kernel.last_results = res
    return out[:n].astype(np.float32)


kernel.last_exec_time_ns = None
